# revision 30
# baseline (speedup 1.0000x reference)
"""Self-contained Trainium2 Bass kernel for causal MHA.

Problem: B=2, S=2048, D=1024, H=16 heads of dim 64, fp32, causal softmax.
  out = softmax(mask(QK^T/8)) V W_0 + b_0 with QKV = X W_qkv + b_qkv.

Sharding: 8 NeuronCores = 2 batches x 4 head-groups (4 heads each),
tensor-parallel over heads, data-parallel over batch. Each core computes a
partial output projection for its 4 heads; host sums the 4 partials per
batch and adds the (bias-folded) output bias.

Device program per core (matmuls in fp32r = full-rate TF32-like mode):
  P1  q^T/k^T = Wqk^T X^T per 512-wide q round (q rows stream per-round in
      qT, k rows persist in kT), +bq on q rows (k bias cancels in softmax).
  P2  V'' [s, 4*65] in bf16: V natural per head plus a ones column, so the
      attn.V matmul also produces softmax row sums for free.
  P3  per head, per 512-wide q superblock: S^T[k,q] = K^T.T @ Q^T computed
      transposed (no on-chip transposes anywhere); fully-masked 128-col
      prefixes of diagonal strips are skipped outright and the triangular
      mask is added only over the 128-wide diagonal block via bf16 factor
      matrices inside the matmul accumulation group; exp on ScalarE
      (scale=1/8), fused over j-pairs into one [128,1024] activation when
      no prefix skip -> P^T (bf16); ctx^T[65,512] += V''^T P^T over k
      blocks.  The ctx matmuls are DEFERRED three j-pair groups behind the
      score matmuls (FIFO pump) so the PE never waits on ScalarE's exp:
      the exp of group g completes while PE runs groups g+1..g+3.  PE
      stalls reset the DVFS ramp (2.4GHz after ~3us continuous, else
      1.2GHz), so a stall-free PE stream runs ~2x faster per matmul.
      Row 64 of ctx^T holds softmax denominators; normalize (emitted
      inline as soon as a head's last ctx group is pumped) via K=1
      broadcast matmul + approx reciprocal + multiply.
  P4  out[s,d] = ctxT.T @ W0 -> DRAM, spread between p3 heads so PE has
      fill work at head boundaries while ScalarE catches up on exp.
DMA queue plan: sync carries ALL input streams (weights, then the xt round
stream, prefetched one round ahead) so the ACT queue carries only exps;
outputs alternate sync/gpsimd.  PSUM tags: 2x sc (paired scores,
2 banks each), 2x strip (projections + normalize broadcast), 2x ctx
(per-head accumulators).
"""
from collections import deque
from contextlib import ExitStack

import numpy as np

import concourse.bass as bass
import concourse.mybir as mybir
import concourse.tile as tile
from concourse import bacc
from concourse.bass_utils import run_bass_kernel_spmd

F32 = mybir.dt.float32
F32R = mybir.dt.float32r
BF16 = mybir.dt.bfloat16
EXP = mybir.ActivationFunctionType.Exp
COPY = mybir.ActivationFunctionType.Copy

S, D, H, HD = 2048, 1024, 16, 64
HG = 4        # heads per core
NB = 4        # 512-wide q superblocks
KC = 8        # contraction chunks of 128 over D
NEG = -1e30
DEPTH = 3     # ctx-matmul deferral depth, in j-pair groups


def _emit(tc, io):
    nc = tc.nc
    with ExitStack() as ctx:
        sb = ctx.enter_context(tc.tile_pool(name="sb", bufs=1))
        ps = ctx.enter_context(tc.tile_pool(name="ps", bufs=2, space="PSUM"))
        wk = ctx.enter_context(tc.tile_pool(name="wk", bufs=2))

        # ---- input DMAs, all on the sync HWDGE queue in consumption
        # order.  The ACT queue carries only exp activations (it is the
        # second-busiest engine); outputs alternate sync/gpsimd.
        wqk_sb = sb.tile([128, KC, 512], BF16, tag="wqk", bufs=2)
        wqk_r = io["wqk"].rearrange("(kc p) n -> kc p n", p=128)
        xt_r = io["xt"].rearrange("(kc p) s -> kc p s", p=128)
        bq_sb = sb.tile([128, 2], F32, tag="bq")
        xtr0 = sb.tile([128, KC, 512], BF16, tag="xt", bufs=2)
        for half in range(2):
            # interleave wqk and first-round xt half-streams so the
            # cold-start p1 can begin once the first halves land
            ks = slice(half * 4, half * 4 + 4)
            nc.sync.dma_start(
                out=wqk_sb[:, ks, :],
                in_=wqk_r[ks].rearrange("kc p n -> p kc n"))
            nc.sync.dma_start(
                out=xtr0[:, ks, :],
                in_=xt_r[ks, :, 0:512].rearrange("kc p n -> p kc n"))
        nc.sync.dma_start(out=bq_sb, in_=io["bq2"])
        # triangular-factor causal mask (bf16): atri.T @ btri[t] adds
        # -1e30*(k_local > q_local) to a diagonal scores tile.
        atri_sb = sb.tile([128, 128], BF16, tag="atri")
        nc.sync.dma_start(out=atri_sb, in_=io["atri"])
        btri_sb = sb.tile([128, 4, 512], BF16, tag="btri")
        nc.sync.dma_start(out=btri_sb, in_=io["btri"])
        wv_sb = sb.tile([128, KC, 256], BF16, tag="wv", bufs=2)
        wv_r = io["wv"].rearrange("(kc p) n -> kc p n", p=128)
        nc.sync.dma_start(
            out=wv_sb, in_=wv_r.rearrange("kc p n -> p kc n"))
        w0_sb = sb.tile([128, 2, D], BF16, tag="w0")
        w0_r = io["w0"].rearrange("(t p) n -> t p n", p=128)
        nc.sync.dma_start(
            out=w0_sb, in_=w0_r.rearrange("t p n -> p t n"))

        # k rows (2 strips) persist the whole iteration; q rows stream
        # per-round.  bufs=2 so the next iteration's projections can start
        # under this iteration's attention tail.
        kT = sb.tile([128, 2, S], BF16, tag="kT", bufs=2)
        vv = sb.tile([128, 16, HG * 65], BF16, tag="vv", bufs=2)
        ctxT = sb.tile([128, 2, S], BF16, tag="ctxT")
        ones1 = sb.tile([1, 64], F32R, tag="ones1")
        ones1f = sb.tile([1, 64], F32, tag="ones1f")
        nc.vector.memset(ones1f, 1.0)
        nc.vector.tensor_copy(ones1, ones1f)
        # softmax-denominator ones columns (col 64 of each head chunk), one
        # strided copy for the whole iteration
        ones_col = sb.tile([128, 16 * HG, 1], F32, tag="onescol")
        nc.vector.memset(ones_col, 1.0)
        nc.vector.tensor_copy(
            vv.rearrange("p s (h c) -> p (s h) c", c=65)[:, :, 64:65], ones_col)

        def load_xt(r):
            xtr = sb.tile([128, KC, 512], BF16, tag="xt", bufs=2)
            for kc in range(KC):
                nc.sync.dma_start(
                    out=xtr[:, kc, :],
                    in_=xt_r[kc, :, r * 512:(r + 1) * 512])
            return xtr

        def p1_tile(xtr, qT, t, n):
            p1 = ps.tile([128, 512], F32, tag="strip", bufs=2)
            for kc in range(KC):
                nc.tensor.matmul(
                    p1,
                    lhsT=wqk_sb[:, kc, t * 128:(t + 1) * 128],
                    rhs=xtr[:, kc, :],
                    start=(kc == 0), stop=(kc == KC - 1))
            if t < 2:
                nc.vector.tensor_scalar_add(
                    qT[:, t, :], p1, bq_sb[:, t:t + 1])
            else:
                # plain copy rides the ACT engine (idle during the p1
                # stretch) so the 2-bank strip rotation never waits on a
                # backed-up DVE queue
                nc.scalar.activation(
                    kT[:, t - 2, n * 512:(n + 1) * 512], p1, COPY)

        def p2_tile(xtr, si):
            p2 = ps.tile([128, 512], F32, tag="strip", bufs=2)
            pp = p2[:, 0:256]
            for kc in range(KC):
                nc.tensor.matmul(
                    pp,
                    lhsT=xtr[:, kc, (si % 4) * 128:(si % 4 + 1) * 128],
                    rhs=wv_sb[:, kc, :],
                    start=(kc == 0), stop=(kc == KC - 1))
            vsl = vv[:, si, :].rearrange("p (h c) -> p h c", c=65)
            nc.scalar.activation(
                vsl[:, :, 0:64], pp.rearrange("p (h c) -> p h c", c=64), COPY)

        def p3_norm_a(cps, h, sbk):
            # stage A: lift the row sums (psum row 64) into SBUF for the
            # broadcast matmul.  Emitted as soon as the head's last ctx
            # group is pumped; stage B follows two pumps later so the DVE
            # copy completes before the PE reaches the broadcast matmul.
            sm = wk.tile([1, 512], F32R, tag="sm", bufs=3)
            nc.vector.tensor_copy(sm, cps[64:65, :])
            return sm

        def p3_norm_b(cps, h, sbk, sm):
            # normalize by row sums: broadcast via K=1 matmul, ~18-bit
            # approximate reciprocal (5x faster than exact; denoms are sums
            # of exps in [1, ~4e3], far from the undefined edges).
            p0 = (h % 2) * 64
            bcp = ps.tile([64, 512], F32, tag="strip", bufs=2)
            nc.tensor.matmul(bcp, lhsT=ones1, rhs=sm, start=True, stop=True)
            rc = wk.tile([64, 512], F32, tag="rc", bufs=3)
            nc.vector.reciprocal_approx_fast(out=rc, in_=bcp)
            nc.vector.tensor_mul(
                ctxT[p0:p0 + 64, h // 2, sbk * 512:(sbk + 1) * 512],
                cps[0:64, :], rc)

        def p4_tile(si, nn):
            po = ps.tile([128, 512], F32, tag="strip", bufs=2)
            for t in range(2):
                nc.tensor.matmul(
                    po,
                    lhsT=ctxT[:, t, si * 128:(si + 1) * 128],
                    rhs=w0_sb[:, t, nn * 512:(nn + 1) * 512],
                    start=(t == 0), stop=(t == 1))
            ob = wk.tile([128, 512], BF16, tag="ob", bufs=6)
            # split the psum->sbuf copies between ACT and DVE so neither
            # queue's backlog stalls the 2-bank strip rotation
            if nn == 0:
                nc.scalar.activation(ob, po, COPY)
            else:
                nc.vector.tensor_copy(ob, po)
            # alternate SWDGE/HWDGE: halves the tail's descriptor-gen
            # serialization.
            eng = nc.sync if (2 * si + nn) % 2 else nc.gpsimd
            eng.dma_start(
                out=io["out"][si * 128:(si + 1) * 128,
                              nn * 512:(nn + 1) * 512],
                in_=ob)

        # ---- deferred ctx-matmul pipeline.  Score/exp groups are emitted
        # eagerly; each group's attn.V matmuls are pumped out DEPTH groups
        # later so the exp result is ready when the PE reaches them.  A
        # head's normalize is emitted inline as soon as its last group is
        # pumped (this keeps the 2-bank cps rotation safe by construction).
        cq = deque()
        left = {}
        nb_pending = []

        def emit_cps(e):
            cps, h, r, pt, jp, v0 = e
            for u in range(2):
                j = jp * 2 + u
                nc.tensor.matmul(
                    cps[:, v0[u]:],
                    lhsT=vv[:, j, :].rearrange(
                        "p (h c) -> p h c", c=65)[:, h, :],
                    rhs=pt[:, u * 512 + v0[u]:(u + 1) * 512],
                    start=(j == 0), stop=(j == 4 * r + 3))
            for nb in nb_pending:
                nb[0] -= 1
            while nb_pending and nb_pending[0][0] <= 0:
                p3_norm_b(*nb_pending.pop(0)[1])
            left[(r, h)] -= 1
            if left[(r, h)] == 0:
                sm = p3_norm_a(cps, h, r)
                nb_pending.append([2, (cps, h, r, sm)])

        def pump_to(limit):
            while len(cq) > limit:
                emit_cps(cq.popleft())

        def pop_one():
            if cq:
                emit_cps(cq.popleft())

        def flush_norms():
            while nb_pending:
                p3_norm_b(*nb_pending.pop(0)[1])

        xtr_cur = xtr0
        for r in range(NB):
            xtr = xtr_cur
            # q/k strips in bf16: same matmul rate at free>=256, but 4x
            # faster on the <256-free diagonal partials, and half-size
            # ldweights streams that hide far better under short matmuls
            qT = sb.tile([128, 2, 512], BF16, tag="qT", bufs=2)
            for t in (0, 2, 1, 3):
                p1_tile(xtr, qT, t, r)
                pop_one()
            for si in range(4 * r, 4 * r + 4):
                p2_tile(xtr, si)
                pop_one()
            if r + 1 < NB:
                xtr_cur = load_xt(r + 1)
            # all of round r-1's normalizes must land before its output
            # projection tiles read ctxT (the pop-countdown can strand the
            # last head's stage-B once the pump queue drains)
            flush_norms()
            for h in range(HG):
                t_q = h // 2
                p0 = (h % 2) * 64
                cps = ps.tile([65, 512], F32, tag="ctx")
                left[(r, h)] = 2 * (r + 1)
                for jp in range(2 * (r + 1)):
                    # pump BEFORE this group's scores: the pumped ctx
                    # matmuls sit in front of the score group leader whose
                    # ldweights carries the psum-bank-reuse wait on exp,
                    # giving that exp ~400ns more to finish (stall -> 0)
                    pump_to(DEPTH)
                    v0 = tuple(
                        max(0, (jp * 2 + u) - 4 * r) * 128 for u in range(2))
                    pt = wk.tile([128, 1024], BF16, tag="pt", bufs=6)
                    sc = ps.tile([128, 1024], F32, tag="sc", bufs=2)
                    for u in range(2):
                        j = jp * 2 + u
                        scu = sc[:, u * 512 + v0[u]:(u + 1) * 512]
                        tt = j - 4 * r
                        diag = tt >= 0
                        nc.tensor.matmul(
                            scu,
                            lhsT=kT[p0:p0 + 64, t_q, j * 128:(j + 1) * 128],
                            rhs=qT[p0:p0 + 64, t_q, v0[u]:512],
                            start=True, stop=not diag)
                        if diag:
                            # triangular mask add only spans the 128-wide
                            # diagonal block
                            nc.tensor.matmul(
                                sc[:, u * 512 + v0[u]:u * 512 + v0[u] + 128],
                                lhsT=atri_sb,
                                rhs=btri_sb[:, tt, tt * 128:(tt + 1) * 128],
                                start=False, stop=True)
                    if v0 == (0, 0):
                        # one fused [128,1024] exp across both psum banks
                        nc.scalar.activation(pt, sc, EXP, scale=0.125)
                    else:
                        for u in range(2):
                            nc.scalar.activation(
                                pt[:, u * 512 + v0[u]:(u + 1) * 512],
                                sc[:, u * 512 + v0[u]:(u + 1) * 512],
                                EXP, scale=0.125)
                    cq.append((cps, h, r, pt, jp, v0))
                    if r > 0 and jp in (1, 3):
                        # spread the previous round's output projection
                        # INSIDE the group stream: widens the PE window the
                        # next group leader's exp dependency must fit in
                        p4_tile(4 * (r - 1) + h, jp // 2)
        pump_to(0)
        flush_norms()
        for si in range(12, 16):
            for nn in range(2):
                p4_tile(si, nn)


def _declare_io(nc):
    return {
        "xt": nc.dram_tensor("xt", [D, S], BF16, kind="ExternalInput")[:, :],
        "wqk": nc.dram_tensor("wqk", [D, 512], BF16,
                              kind="ExternalInput")[:, :],
        "bq2": nc.dram_tensor("bq2", [128, 2], F32,
                              kind="ExternalInput")[:, :],
        "wv": nc.dram_tensor("wv", [D, 256], BF16, kind="ExternalInput")[:, :],
        "w0": nc.dram_tensor("w0", [256, D], BF16, kind="ExternalInput")[:, :],
        "atri": nc.dram_tensor("atri", [128, 128], BF16,
                               kind="ExternalInput")[:, :],
        "btri": nc.dram_tensor("btri", [128, 4, 512], BF16,
                               kind="ExternalInput")[:, :, :],
        "out": nc.dram_tensor("out", [S, D], BF16,
                              kind="ExternalOutput")[:, :],
    }


_NC_CACHE = {}


def _build():
    if "nc" not in _NC_CACHE:
        nc = bacc.Bacc("TRN2", target_bir_lowering=False, debug=False,
                       num_devices=8)
        io = _declare_io(nc)
        with tile.TileContext(nc) as tc:
            _emit(tc, io)
        nc.compile()
        _NC_CACHE["nc"] = nc
    return _NC_CACHE["nc"]


def _causal_mask_factors():
    import ml_dtypes
    k = np.arange(128)[:, None]
    p = np.arange(128)[None, :]
    f = np.arange(512)[None, :]
    a = (k <= p).astype(ml_dtypes.bfloat16)
    b = np.zeros((128, 4, 512), ml_dtypes.bfloat16)
    for t in range(4):
        b[:, t, :] = np.where(k > f - t * 128, NEG, 0.0).astype(
            ml_dtypes.bfloat16)
    return a, b


def _core_inputs(X, W_qkv, b_qkv, W_0):
    import ml_dtypes
    bf = ml_dtypes.bfloat16
    atri, btri = _causal_mask_factors()
    maps = []
    for c in range(8):
        b, g = divmod(c, 4)
        cs = slice(g * 256, (g + 1) * 256)
        wqk = np.concatenate(
            [W_qkv[:, g * 256:(g + 1) * 256],
             W_qkv[:, 1024 + g * 256:1024 + (g + 1) * 256]], axis=1)
        maps.append({
            "xt": np.ascontiguousarray(X[b].T).astype(bf),
            "wqk": np.ascontiguousarray(wqk).astype(bf),
            "bq2": np.ascontiguousarray(b_qkv[cs].reshape(2, 128).T),
            "wv": np.ascontiguousarray(
                W_qkv[:, 2048 + g * 256:2048 + (g + 1) * 256]).astype(bf),
            "w0": np.ascontiguousarray(W_0[cs, :]).astype(bf),
            "atri": atri,
            "btri": btri,
        })
    return maps


def kernel(X, W_qkv, b_qkv, W_0, b_0):
    X = np.asarray(X, np.float32)
    W_qkv = np.asarray(W_qkv, np.float32)
    b_qkv = np.asarray(b_qkv, np.float32)
    W_0 = np.asarray(W_0, np.float32)
    b_0 = np.asarray(b_0, np.float32)

    nc = _build()
    maps = _core_inputs(X, W_qkv, b_qkv, W_0)
    res = run_bass_kernel_spmd(nc, maps, core_ids=list(range(8))).results

    bias = b_qkv[2048:] @ W_0 + b_0   # V-bias folded (softmax rows sum to 1)
    out = np.zeros((2, S, D), np.float32)
    for c in range(8):
        out[c // 4] += res[c]["out"].astype(np.float32)
    out += bias[None, None, :]
    return out
